# revision 35
# baseline (speedup 1.0000x reference)
"""Trainium2 Bass kernel for nn_AttentionBlock (B=4, N=1024, C=1024, H=16).

Sharding: 8 cores = 4 batches x 2 head-groups (8 heads each). Each core
computes its batch's tokens for its 8 heads end-to-end; the host sums the
two partial output projections per batch.

Key optimizations over the v1 kernel:
- All matmul operands in bf16 (psum accumulation stays f32); inputs are
  cast host-side, halving HBM traffic and SBUF footprint.
- LayerNorm mean subtraction is folded into the QKV weights host-side
  (centering each head's weight columns is an exact identity for LN), so
  the device only computes sum(x^2) -> rstd -> one multiply.
- rstd = exp(-0.5*ln(var+eps)) so every ACT function used (Square, Ln,
  Exp, Copy) lives in one activation table -> a single table load.
- q/k [token, dh] -> [dh, token] transposes run on the DMA xbar
  (dma_start_transpose), not the PE array.
- Softmax normalization: reciprocal of the appended-ones row, then a
  gpsimd partition_broadcast (no DRAM round trip).
- Full weight matrix cached in SBUF; w is streamed exactly once.
- Scores for a head pair run concurrently in the PE via tile_position
  row groups; exp processes two heads per ACT op ([128,1024] psum tile).
- Output projection interleaved with the second half of attention.
"""
import os
import sys

sys.path.insert(0, "/opt/trn_rl_repo")

import numpy as np
import ml_dtypes

import functools

import concourse.bass as bass
import concourse.bacc as bacc
import concourse.tile as tile
from concourse import mybir
from concourse.bass_utils import run_bass_kernel_spmd
from concourse.hw_specs import get_activation_tables as _real_act_tables

_AF = mybir.ActivationFunctionType


@functools.cache
def _one_table_act_sets(arch):
    """All ACT functions this kernel uses (Square, Ln, Exp) live together in
    the natural_log_exp_and_others table.  The table-load insertion pass
    assigns each activation the first set containing its function, which
    makes Square/Exp (set 0) and Ln (set 5) thrash 30+ table loads.  Strip
    those functions from every other set so each activation resolves to the
    shared table; set indices are preserved, so walrus' id mapping and the
    on-device tables are unchanged."""
    tabs = _real_act_tables(arch)
    target = "natural_log_exp_and_others"
    if target not in tabs or not {_AF.Square, _AF.Ln, _AF.Exp} <= tabs[target]:
        return tabs
    mine = {_AF.Square, _AF.Ln, _AF.Exp, _AF.Copy, _AF.Identity}
    return {
        name: set(s) if name == target else (set(s) - mine)
        for name, s in tabs.items()
    }


bacc.get_activation_tables = _one_table_act_sets

F32 = mybir.dt.float32
BF16 = mybir.dt.bfloat16
NPBF16 = ml_dtypes.bfloat16

B, N, C, H = 4, 1024, 1024, 16
DH = C // H            # 64
HPC = 8                # heads per core
NT = N // 128          # 8 token tiles
KC = (2 * C) // 128    # 16 contraction chunks for fused qkv+dt
EPS = 1e-5
AX = mybir.AxisListType.X
ALU = mybir.AluOpType
AF = mybir.ActivationFunctionType


def _bcast_free(ap, n, axis_pos=1):
    """Insert a step-0 free dim of size n at axis_pos of an AP."""
    new = list(ap.ap)
    new.insert(axis_pos, [0, n])
    return bass.AP(tensor=ap.tensor, offset=ap.offset, ap=new)


def _bcast_part(ap, n):
    """Partition-broadcast AP (step-0 partition dim) for DMA use."""
    return bass.AP(tensor=ap.tensor, offset=ap.offset, ap=[[0, n]] + list(ap.ap[1:]))


def build(l1, debug=False):
    """Build the single-core SPMD program. l1: python float (lamb1).
    lamb2 and the v-bias are folded into the prescaled vres input."""
    nc = bacc.Bacc("TRN2", target_bir_lowering=False)

    xdT = nc.dram_tensor("xdT", [2 * C, N], BF16, kind="ExternalInput")
    w = nc.dram_tensor("w", [2 * C, 3 * HPC * DH], BF16, kind="ExternalInput")
    vres = nc.dram_tensor("vres", [N, HPC * DH], F32, kind="ExternalInput")
    wproj = nc.dram_tensor("wproj", [HPC * DH, C], BF16, kind="ExternalInput")
    rope = nc.dram_tensor("rope", [N, 2 * DH], BF16, kind="ExternalInput")
    out = nc.dram_tensor("out", [N, C], F32, kind="ExternalOutput")
    dbg = {}
    if debug:
        for nm, shp in [("d_qT", [128, 4, N]), ("d_kT", [128, 4, N]),
                        ("d_v", [128, NT * HPC * (DH + 1)]),
                        ("d_oT", [128, 4, N]), ("d_rcp", [16, 512]),
                        ("d_sum", [16, 512])]:
            dbg[nm] = nc.dram_tensor(
                nm, shp, BF16 if nm in ("d_qT", "d_kT", "d_v", "d_oT") else F32,
                kind="ExternalOutput")

    with tile.TileContext(nc) as tc:
        with tc.tile_pool(name="longp", bufs=1) as longp:
            eps_t = longp.tile([128, 1], F32)
            nc.vector.memset(eps_t, EPS)
            xdT_sb = longp.tile([128, KC, N], BF16)
            w_sb = longp.tile([128, KC, 3 * HPC * DH], BF16)
            rp_sb = longp.tile([128, NT, 2 * DH], BF16)
            wproj_sb = longp.tile([128, 4, C], BF16)
            v_sb = longp.tile([128, NT, HPC, DH + 1], BF16)
            qT_sb = longp.tile([128, HPC // 2, N], BF16)
            kT_sb = longp.tile([128, HPC // 2, N], BF16)
            outT_sb = longp.tile([128, HPC // 2, N], BF16)

            # bulk loads spread across the three DMA-capable queues, ordered
            # kc-ascending so the t=0 contraction can start streaming early
            for kc in range(KC):
                weng = nc.gpsimd if kc % 2 == 0 else nc.scalar
                weng.dma_start(out=w_sb[:, kc, :],
                               in_=w[kc * 128:(kc + 1) * 128, :])
                xeng = nc.sync if kc % 2 == 0 else nc.gpsimd
                xeng.dma_start(out=xdT_sb[:, kc, :],
                               in_=xdT[kc * 128:(kc + 1) * 128, :])
            # after ALL x/w chunks: phase A needs every kc chunk before its
            # first tile completes, rope/wproj only much later
            nc.sync.dma_start(out=rp_sb,
                              in_=rope[:, :].rearrange("(t p) d -> p t d", p=128))
            for cc in range(4):
                nc.sync.dma_start(out=wproj_sb[:, cc, :],
                                  in_=wproj[cc * 128:(cc + 1) * 128, :])
            # ones column of v (row DH of each head's stationary block)
            nc.vector.memset(v_sb[:, :, :, DH:DH + 1], 1.0)

            # ---------------- phase A: fused qkv+dt projection ----------------
            with (
                tc.tile_pool(name="vresp", bufs=2) as vresp,
                tc.tile_pool(name="sqp", bufs=2) as sqp,
                tc.tile_pool(name="redp", bufs=3) as redp,
                tc.tile_pool(name="qnp", bufs=2) as qnp,
                tc.tile_pool(name="qrp", bufs=3) as qrp,
                tc.tile_pool(name="tmpp", bufs=2) as tmpp,
                tc.tile_pool(name="psA", bufs=2, space="PSUM") as psA,
            ):
                for t in range(NT):
                    vt = vresp.tile([128, HPC * DH], F32, tag="vt")
                    nc.gpsimd.dma_start(out=vt, in_=vres[t * 128:(t + 1) * 128, :])
                    ps = psA.tile([128, 3 * HPC * DH], F32, tag="ps")
                    for kc in range(KC):
                        stat = xdT_sb[:, kc, t * 128:(t + 1) * 128]
                        for ob in range(3):
                            nc.tensor.matmul(
                                ps[:, ob * 512:(ob + 1) * 512],
                                stat,
                                w_sb[:, kc, ob * 512:(ob + 1) * 512],
                                start=(kc == 0), stop=(kc == KC - 1))
                    # ---- q / k: rstd, scale, rope, transpose ----
                    for qk in range(2):
                        pslice = ps[:, qk * 512:(qk + 1) * 512]
                        ps3 = pslice.rearrange("p (h d) -> p h d", h=HPC)
                        sq = sqp.tile([128, HPC * DH], BF16, tag="sq")
                        nc.scalar.activation(out=sq[:], in_=pslice, func=AF.Square)
                        red = redp.tile([128, HPC], F32, tag="red")
                        nc.vector.reduce_sum(
                            out=red[:],
                            in_=sq.rearrange("p (h d) -> p h d", h=HPC), axis=AX)
                        # rstd = exp(-0.5*ln(var/DH + eps)); Ln+Exp share the
                        # softmax Exp's ACT table, so no table reloads.
                        lnv = redp.tile([128, HPC], F32, tag="lnv")
                        nc.scalar.activation(out=lnv[:], in_=red[:], func=AF.Ln,
                                             scale=1.0 / DH, bias=eps_t[:])
                        rstd = redp.tile([128, HPC], F32, tag="rstd")
                        nc.scalar.activation(out=rstd[:], in_=lnv[:], func=AF.Exp,
                                             scale=-0.5)
                        qn = qnp.tile([128, HPC, DH], BF16, tag="qn")
                        nc.vector.tensor_tensor(
                            out=qn[:], in0=ps3,
                            in1=_bcast_free(rstd[:], DH, 2)[:], op=ALU.mult)
                        # rope: out_lo = lo*cos0 - hi*sin0; out_hi = hi*cos1 + lo*sin1
                        HD = DH // 2
                        sin0 = _bcast_free(rp_sb[:, t, 0:HD], HPC, 1)
                        sin1 = _bcast_free(rp_sb[:, t, HD:DH], HPC, 1)
                        cos0 = _bcast_free(rp_sb[:, t, DH:DH + HD], HPC, 1)
                        cos1 = _bcast_free(rp_sb[:, t, DH + HD:2 * DH], HPC, 1)
                        qr = qrp.tile([128, HPC, DH], BF16, tag="qr")
                        t1 = tmpp.tile([128, HPC, HD], BF16, tag="t1")
                        t2 = tmpp.tile([128, HPC, HD], BF16, tag="t2")
                        lo = qn[:, :, 0:HD]
                        hi = qn[:, :, HD:DH]
                        nc.vector.tensor_tensor(out=t1[:], in0=hi, in1=sin0[:],
                                                op=ALU.mult)
                        nc.vector.tensor_tensor(out=t2[:], in0=lo, in1=cos0[:],
                                                op=ALU.mult)
                        nc.vector.tensor_tensor(out=qr[:, :, 0:HD],
                                                in0=t2[:], in1=t1[:],
                                                op=ALU.subtract)
                        nc.vector.tensor_tensor(out=t1[:], in0=lo, in1=sin1[:],
                                                op=ALU.mult)
                        nc.vector.tensor_tensor(out=t2[:], in0=hi, in1=cos1[:],
                                                op=ALU.mult)
                        nc.vector.tensor_tensor(out=qr[:, :, HD:DH],
                                                in0=t2[:], in1=t1[:],
                                                op=ALU.add)
                        # one batched xbar transpose per (t, q/k):
                        # [128 tok, 512] -> [128 (hh,dh), 4 j, 128 tok]
                        dst = qT_sb if qk == 0 else kT_sb
                        eng = nc.sync if qk == 0 else nc.scalar
                        eng.dma_start_transpose(
                            out=dst[:, :, t * 128:(t + 1) * 128],
                            in_=qr.rearrange("p h d -> p (h d)"))
                    # ---- v = l1*(v+dv) + pre-scaled residual ----
                    nc.vector.scalar_tensor_tensor(
                        out=v_sb[:, t, :, 0:DH],
                        in0=ps[:, 1024:1536].rearrange("p (h d) -> p h d", h=HPC),
                        scalar=float(l1),
                        in1=vt.rearrange("p (h d) -> p h d", h=HPC),
                        op0=ALU.mult, op1=ALU.add)

            if debug:
                nc.sync.dma_start(out=dbg["d_qT"][:, :, :], in_=qT_sb)
                nc.sync.dma_start(out=dbg["d_kT"][:, :, :], in_=kT_sb)
                nc.sync.dma_start(
                    out=dbg["d_v"][:, :],
                    in_=v_sb.rearrange("p a b c -> p (a b c)"))
            # ---------------- attention + output projection ----------------
            with (
                tc.tile_pool(name="exp", bufs=3) as exp_p,
                tc.tile_pool(name="rcpp", bufs=2) as rcpp,
                tc.tile_pool(name="rcbp", bufs=2) as rcbp,
                tc.tile_pool(name="ostg", bufs=2) as ostg,
                tc.tile_pool(name="stgp", bufs=2) as stgp,
                tc.tile_pool(name="psS", bufs=2, space="PSUM") as psS,
                tc.tile_pool(name="psV", bufs=2, space="PSUM") as psV,
                tc.tile_pool(name="psP", bufs=2, space="PSUM") as psP,
            ):
                def emit_sc(j, qh, kc):
                    sc = psS.tile([128, 1024], F32, tag="sc", name="sc")
                    nc.tensor.matmul(
                        sc[:, 0:512],
                        kT_sb[0:DH, j, kc * 128:(kc + 1) * 128],
                        qT_sb[0:DH, j, qh * 512:(qh + 1) * 512],
                        start=True, stop=True, tile_position=(0, 0))
                    nc.tensor.matmul(
                        sc[:, 512:1024],
                        kT_sb[DH:128, j, kc * 128:(kc + 1) * 128],
                        qT_sb[DH:128, j, qh * 512:(qh + 1) * 512],
                        start=True, stop=True, tile_position=(DH, 0))
                    return sc

                def attn_iter(j, qh):
                    avA = psV.tile([DH + 1, 512], F32, tag="av")
                    avB = psV.tile([DH + 1, 512], F32, tag="av")
                    # software-pipelined by one chunk: sc(kc+1) is emitted
                    # BEFORE av(kc) so the in-order PE queue keeps the next
                    # score matmul ahead of the exp-gated attend matmul and
                    # the scalar engine streams exps back-to-back
                    sc = emit_sc(j, qh, 0)
                    for kc in range(NT):
                        ex = exp_p.tile([128, 1024], BF16, tag="ex")
                        nc.scalar.activation(out=ex[:], in_=sc[:], func=AF.Exp,
                                             scale=1.0 / float(np.sqrt(DH)))
                        if kc < NT - 1:
                            sc = emit_sc(j, qh, kc + 1)
                        nc.tensor.matmul(avA[:], v_sb[:, kc, 2 * j, :],
                                         ex[:, 0:512],
                                         start=(kc == 0), stop=(kc == NT - 1))
                        nc.tensor.matmul(avB[:], v_sb[:, kc, 2 * j + 1, :],
                                         ex[:, 512:1024],
                                         start=(kc == 0), stop=(kc == NT - 1))
                    for hh, av in ((0, avA), (1, avB)):
                        # stage av to SBUF immediately: frees the PSUM bank
                        # for the next iteration, and the approx-reciprocal
                        # custom op misreads PSUM operands on hardware
                        avs = rcpp.tile([DH + 1, 512], F32, tag="avs")
                        nc.vector.tensor_copy(avs[:], av[:])
                        # custom DVE ops and partition_broadcast only honor
                        # partition base 0 on HW: DMA the sum row down first
                        s0 = rcpp.tile([1, 512], F32, tag="s0")
                        nc.sync.dma_start(out=s0[:, :], in_=avs[DH:DH + 1, :])
                        rcp = rcpp.tile([1, 512], F32, tag="rcp")
                        nc.vector.reciprocal_approx_fast(
                            out=rcp[:], in_=s0[:])
                        if debug:
                            ri = (4 * qh + j) * 2 + hh
                            nc.sync.dma_start(
                                out=dbg["d_sum"][ri:ri + 1, :], in_=s0[:, :])
                            nc.sync.dma_start(
                                out=dbg["d_rcp"][ri:ri + 1, :], in_=rcp[:, :])
                        rcb = rcbp.tile([DH, 512], F32, tag="rcb")
                        nc.gpsimd.partition_broadcast(rcb[:], rcp[:])
                        if hh == 0:
                            nc.vector.tensor_tensor(
                                out=outT_sb[0:DH, j, qh * 512:(qh + 1) * 512],
                                in0=avs[0:DH, :], in1=rcb[:], op=ALU.mult)
                        else:
                            # keep DVE partition bases aligned: normalize at
                            # base 0, then DMA the rows into place
                            ob_stg = ostg.tile([DH, 512], BF16, tag="ob")
                            nc.vector.tensor_tensor(
                                out=ob_stg[:], in0=avs[0:DH, :], in1=rcb[:],
                                op=ALU.mult)
                            nc.sync.dma_start(
                                out=outT_sb[DH:128, j, qh * 512:(qh + 1) * 512],
                                in_=ob_stg[:])

                def proj_t(t, on_act=False):
                    stg = stgp.tile([128, C], F32, tag="stg")
                    for oh in range(2):
                        pp = psP.tile([128, 512], F32, tag="pp")
                        for cc in range(4):
                            nc.tensor.matmul(
                                pp[:],
                                outT_sb[:, cc, t * 128:(t + 1) * 128],
                                wproj_sb[:, cc, oh * 512:(oh + 1) * 512],
                                start=(cc == 0), stop=(cc == 3))
                        if on_act:
                            # tail projections: the scalar engine is idle
                            # after the last exp
                            nc.scalar.copy(
                                out=stg[:, oh * 512:(oh + 1) * 512], in_=pp[:])
                        else:
                            nc.vector.tensor_copy(
                                stg[:, oh * 512:(oh + 1) * 512], pp[:])
                    nc.sync.dma_start(out=out[t * 128:(t + 1) * 128, :], in_=stg)

                for j in range(HPC // 2):
                    attn_iter(j, 0)
                for j in range(HPC // 2):
                    attn_iter(j, 1)
                    proj_t(j)
                for t in range(4, NT):
                    proj_t(t, on_act=True)
                if debug:
                    nc.sync.dma_start(out=dbg["d_oT"][:, :, :], in_=outT_sb)

    nc.finalize()
    return nc


def prepare(x, rope, delta_t_emb, v_residual_v1, Wqkv, bqkv, Wdt, bdt,
            qn_g, qn_b, kn_g, kn_b, lamb1, lamb2, Wproj, bproj):
    """Host-side prep: returns (l1, in_maps, bproj)."""
    x = np.asarray(x, np.float32)
    rope = np.ascontiguousarray(np.asarray(rope, np.float32))
    delta_t_emb = np.asarray(delta_t_emb, np.float32)
    v_residual_v1 = np.asarray(v_residual_v1, np.float32)
    Wqkv = np.asarray(Wqkv, np.float32)
    Wdt = np.asarray(Wdt, np.float32)
    Wproj = np.asarray(Wproj, np.float32)
    bias = np.asarray(bqkv, np.float32) + np.asarray(bdt, np.float32)
    l1 = float(np.asarray(lamb1)); l2 = float(np.asarray(lamb2))
    qn_g = np.asarray(qn_g, np.float32); qn_b = np.asarray(qn_b, np.float32)
    kn_g = np.asarray(kn_g, np.float32); kn_b = np.asarray(kn_b, np.float32)

    assert not np.any(bias[:2 * C]), "nonzero q/k bias not supported"
    assert not np.any(qn_b) and not np.any(kn_b), "LN beta not supported"

    # fold LN gamma into the rope tables (per-channel scale commutes with
    # the elementwise rope rotation when gamma is rotate-half symmetric,
    # i.e. gamma[d] == gamma[(d+DH/2)%DH]; the reference's gamma is all-ones)
    HD = DH // 2
    sym_q = np.allclose(qn_g[:HD], qn_g[HD:])
    sym_k = np.allclose(kn_g[:HD], kn_g[HD:])
    assert sym_q and sym_k, "rotate-half-asymmetric LN gamma not supported"
    # fold gamma into the tables: with rotate-half-symmetric gamma, scaling
    # both the sin and cos halves by gamma equals applying gamma before rope
    rope_q = rope * np.concatenate([qn_g, qn_g])[None, :]
    rope_k = rope * np.concatenate([kn_g, kn_g])[None, :]
    same_rope = np.allclose(rope_q, rope_k)

    in_maps = []
    for c in range(8):
        b = c // 2
        g = c % 2
        rsl = slice(g * 512, (g + 1) * 512)
        w_core = np.concatenate([
            np.concatenate([Wqkv[rsl], Wqkv[C:][rsl], Wqkv[2 * C:][rsl]], 0).T,
            np.concatenate([Wdt[rsl], Wdt[C:][rsl], Wdt[2 * C:][rsl]], 0).T,
        ], axis=0).copy()  # [2048, 1536]; cols = q(512) | k(512) | v(512)
        # center q/k weight columns per head: exact identity under LayerNorm
        for h in range(HPC):
            for blk in range(2):
                cs = blk * 512 + h * DH
                w_core[:, cs:cs + DH] -= w_core[:, cs:cs + DH].mean(
                    axis=1, keepdims=True)
        vres_pre = (v_residual_v1[b, g * 8:(g + 1) * 8]
                    .transpose(1, 0, 2).reshape(N, 512) * l2)
        if np.any(bias[2 * C:]):
            vres_pre = vres_pre + l1 * bias[2 * C:][rsl][None, :]
        m = {
            "xdT": np.ascontiguousarray(
                np.concatenate([x[b].T, delta_t_emb[b].T], 0)).astype(NPBF16),
            "w": np.ascontiguousarray(w_core).astype(NPBF16),
            "vres": np.ascontiguousarray(vres_pre, dtype=np.float32),
            "wproj": np.ascontiguousarray(Wproj[:, rsl].T).astype(NPBF16),
            "rope": np.ascontiguousarray(rope_q).astype(NPBF16),
        }
        assert same_rope, "distinct q/k LN gamma not supported"
        in_maps.append(m)
    return l1, in_maps, np.asarray(bproj, np.float32)


_CACHE = {}
_LAST_RES = None


def kernel(x, rope, delta_t_emb, v_residual_v1, Wqkv, bqkv, Wdt, bdt,
           qn_g, qn_b, kn_g, kn_b, lamb1, lamb2, Wproj, bproj):
    l1, in_maps, bproj = prepare(
        x, rope, delta_t_emb, v_residual_v1, Wqkv, bqkv, Wdt, bdt,
        qn_g, qn_b, kn_g, kn_b, lamb1, lamb2, Wproj, bproj)

    dbgf = bool(int(os.environ.get("KERNEL_DEBUG", "0")))
    key = (l1, dbgf)
    if key not in _CACHE:
        _CACHE[key] = build(l1, debug=dbgf)
    nc = _CACHE[key]

    trace = bool(int(os.environ.get("KERNEL_TRACE", "0")))
    res = run_bass_kernel_spmd(nc, in_maps, core_ids=list(range(8)), trace=trace)
    global _LAST_RES
    _LAST_RES = res
    if trace and res.exec_time_ns is not None:
        print(f"HW exec time: {res.exec_time_ns} ns")
        kernel.last_exec_time_ns = res.exec_time_ns
        kernel.last_results = res

    out = np.empty((B, N, C), np.float32)
    for b in range(B):
        out[b] = res.results[2 * b]["out"] + res.results[2 * b + 1]["out"]
    if np.any(bproj):
        out += bproj[None, None, :]
    return out


# revision 37
# speedup vs baseline: 1.0021x; 1.0021x over previous
"""Trainium2 Bass kernel for nn_AttentionBlock (B=4, N=1024, C=1024, H=16).

Sharding: 8 cores = 4 batches x 2 head-groups (8 heads each). Each core
computes its batch's tokens for its 8 heads end-to-end; the host sums the
two partial output projections per batch.

Key optimizations over the v1 kernel:
- All matmul operands in bf16 (psum accumulation stays f32); inputs are
  cast host-side, halving HBM traffic and SBUF footprint.
- LayerNorm mean subtraction is folded into the QKV weights host-side
  (centering each head's weight columns is an exact identity for LN), so
  the device only computes sum(x^2) -> rstd -> one multiply.
- rstd = exp(-0.5*ln(var+eps)) so every ACT function used (Square, Ln,
  Exp, Copy) lives in one activation table -> a single table load.
- q/k [token, dh] -> [dh, token] transposes run on the DMA xbar
  (dma_start_transpose), not the PE array.
- Softmax normalization: reciprocal of the appended-ones row, then a
  gpsimd partition_broadcast (no DRAM round trip).
- Full weight matrix cached in SBUF; w is streamed exactly once.
- Scores for a head pair run concurrently in the PE via tile_position
  row groups; exp processes two heads per ACT op ([128,1024] psum tile).
- Output projection interleaved with the second half of attention.
"""
import os
import sys

sys.path.insert(0, "/opt/trn_rl_repo")

import numpy as np
import ml_dtypes

import functools

import concourse.bass as bass
import concourse.bacc as bacc
import concourse.tile as tile
from concourse import mybir
from concourse.bass_utils import run_bass_kernel_spmd
from concourse.hw_specs import get_activation_tables as _real_act_tables

_AF = mybir.ActivationFunctionType


@functools.cache
def _one_table_act_sets(arch):
    """All ACT functions this kernel uses (Square, Ln, Exp) live together in
    the natural_log_exp_and_others table.  The table-load insertion pass
    assigns each activation the first set containing its function, which
    makes Square/Exp (set 0) and Ln (set 5) thrash 30+ table loads.  Strip
    those functions from every other set so each activation resolves to the
    shared table; set indices are preserved, so walrus' id mapping and the
    on-device tables are unchanged."""
    tabs = _real_act_tables(arch)
    target = "natural_log_exp_and_others"
    if target not in tabs or not {_AF.Square, _AF.Ln, _AF.Exp} <= tabs[target]:
        return tabs
    mine = {_AF.Square, _AF.Ln, _AF.Exp, _AF.Copy, _AF.Identity}
    return {
        name: set(s) if name == target else (set(s) - mine)
        for name, s in tabs.items()
    }


bacc.get_activation_tables = _one_table_act_sets

F32 = mybir.dt.float32
BF16 = mybir.dt.bfloat16
NPBF16 = ml_dtypes.bfloat16

B, N, C, H = 4, 1024, 1024, 16
DH = C // H            # 64
HPC = 8                # heads per core
NT = N // 128          # 8 token tiles
KC = (2 * C) // 128    # 16 contraction chunks for fused qkv+dt
EPS = 1e-5
AX = mybir.AxisListType.X
ALU = mybir.AluOpType
AF = mybir.ActivationFunctionType


def _bcast_free(ap, n, axis_pos=1):
    """Insert a step-0 free dim of size n at axis_pos of an AP."""
    new = list(ap.ap)
    new.insert(axis_pos, [0, n])
    return bass.AP(tensor=ap.tensor, offset=ap.offset, ap=new)


def _bcast_part(ap, n):
    """Partition-broadcast AP (step-0 partition dim) for DMA use."""
    return bass.AP(tensor=ap.tensor, offset=ap.offset, ap=[[0, n]] + list(ap.ap[1:]))


def build(l1, debug=False):
    """Build the single-core SPMD program. l1: python float (lamb1).
    lamb2 and the v-bias are folded into the prescaled vres input."""
    nc = bacc.Bacc("TRN2", target_bir_lowering=False)

    xdT = nc.dram_tensor("xdT", [2 * C, N], BF16, kind="ExternalInput")
    w = nc.dram_tensor("w", [2 * C, 3 * HPC * DH], BF16, kind="ExternalInput")
    vres = nc.dram_tensor("vres", [N, HPC * DH], F32, kind="ExternalInput")
    wproj = nc.dram_tensor("wproj", [HPC * DH, C], BF16, kind="ExternalInput")
    rope = nc.dram_tensor("rope", [N, 2 * DH], BF16, kind="ExternalInput")
    out = nc.dram_tensor("out", [N, C], F32, kind="ExternalOutput")
    dbg = {}
    if debug:
        for nm, shp in [("d_qT", [128, 4, N]), ("d_kT", [128, 4, N]),
                        ("d_v", [128, NT * HPC * (DH + 1)]),
                        ("d_oT", [128, 4, N]), ("d_rcp", [16, 512]),
                        ("d_sum", [16, 512])]:
            dbg[nm] = nc.dram_tensor(
                nm, shp, BF16 if nm in ("d_qT", "d_kT", "d_v", "d_oT") else F32,
                kind="ExternalOutput")

    with tile.TileContext(nc) as tc:
        with tc.tile_pool(name="longp", bufs=1) as longp:
            eps_t = longp.tile([128, 1], F32)
            nc.vector.memset(eps_t, EPS)
            xdT_sb = longp.tile([128, KC, N], BF16)
            w_sb = longp.tile([128, KC, 3 * HPC * DH], BF16)
            rp_sb = longp.tile([128, NT, 2 * DH], BF16)
            wproj_sb = longp.tile([128, 4, C], BF16)
            v_sb = longp.tile([128, NT, HPC, DH + 1], BF16)
            qT_sb = longp.tile([128, HPC // 2, N], BF16)
            kT_sb = longp.tile([128, HPC // 2, N], BF16)
            outT_sb = longp.tile([128, HPC // 2, N], BF16)

            # bulk loads spread across the three DMA-capable queues, ordered
            # kc-ascending so the t=0 contraction can start streaming early
            for kc in range(KC):
                weng = nc.gpsimd if kc % 2 == 0 else nc.scalar
                weng.dma_start(out=w_sb[:, kc, :],
                               in_=w[kc * 128:(kc + 1) * 128, :])
                xeng = nc.sync if kc % 2 == 0 else nc.gpsimd
                xeng.dma_start(out=xdT_sb[:, kc, :],
                               in_=xdT[kc * 128:(kc + 1) * 128, :])
            # after ALL x/w chunks (phase A needs every kc chunk before its
            # first tile completes, rope/wproj only much later); issue on the
            # scalar queue so sync's xdT chunks are not delayed behind them
            nc.scalar.dma_start(out=rp_sb,
                                in_=rope[:, :].rearrange("(t p) d -> p t d",
                                                         p=128))
            for cc in range(4):
                nc.scalar.dma_start(out=wproj_sb[:, cc, :],
                                    in_=wproj[cc * 128:(cc + 1) * 128, :])
            # ones column of v (row DH of each head's stationary block)
            nc.vector.memset(v_sb[:, :, :, DH:DH + 1], 1.0)

            # ---------------- phase A: fused qkv+dt projection ----------------
            with (
                tc.tile_pool(name="vresp", bufs=2) as vresp,
                tc.tile_pool(name="sqp", bufs=2) as sqp,
                tc.tile_pool(name="redp", bufs=3) as redp,
                tc.tile_pool(name="qnp", bufs=2) as qnp,
                tc.tile_pool(name="qrp", bufs=3) as qrp,
                tc.tile_pool(name="tmpp", bufs=2) as tmpp,
                tc.tile_pool(name="psA", bufs=2, space="PSUM") as psA,
            ):
                for t in range(NT):
                    vt = vresp.tile([128, HPC * DH], F32, tag="vt")
                    nc.gpsimd.dma_start(out=vt, in_=vres[t * 128:(t + 1) * 128, :])
                    ps = psA.tile([128, 3 * HPC * DH], F32, tag="ps")
                    for kc in range(KC):
                        stat = xdT_sb[:, kc, t * 128:(t + 1) * 128]
                        for ob in range(3):
                            nc.tensor.matmul(
                                ps[:, ob * 512:(ob + 1) * 512],
                                stat,
                                w_sb[:, kc, ob * 512:(ob + 1) * 512],
                                start=(kc == 0), stop=(kc == KC - 1))
                    # ---- q / k: rstd, scale, rope, transpose ----
                    for qk in range(2):
                        pslice = ps[:, qk * 512:(qk + 1) * 512]
                        ps3 = pslice.rearrange("p (h d) -> p h d", h=HPC)
                        sq = sqp.tile([128, HPC * DH], BF16, tag="sq")
                        nc.scalar.activation(out=sq[:], in_=pslice, func=AF.Square)
                        red = redp.tile([128, HPC], F32, tag="red")
                        nc.vector.reduce_sum(
                            out=red[:],
                            in_=sq.rearrange("p (h d) -> p h d", h=HPC), axis=AX)
                        # rstd = exp(-0.5*ln(var/DH + eps)); Ln+Exp share the
                        # softmax Exp's ACT table, so no table reloads.
                        lnv = redp.tile([128, HPC], F32, tag="lnv")
                        nc.scalar.activation(out=lnv[:], in_=red[:], func=AF.Ln,
                                             scale=1.0 / DH, bias=eps_t[:])
                        rstd = redp.tile([128, HPC], F32, tag="rstd")
                        nc.scalar.activation(out=rstd[:], in_=lnv[:], func=AF.Exp,
                                             scale=-0.5)
                        qn = qnp.tile([128, HPC, DH], BF16, tag="qn")
                        nc.vector.tensor_tensor(
                            out=qn[:], in0=ps3,
                            in1=_bcast_free(rstd[:], DH, 2)[:], op=ALU.mult)
                        # rope: out_lo = lo*cos0 - hi*sin0; out_hi = hi*cos1 + lo*sin1
                        HD = DH // 2
                        sin0 = _bcast_free(rp_sb[:, t, 0:HD], HPC, 1)
                        sin1 = _bcast_free(rp_sb[:, t, HD:DH], HPC, 1)
                        cos0 = _bcast_free(rp_sb[:, t, DH:DH + HD], HPC, 1)
                        cos1 = _bcast_free(rp_sb[:, t, DH + HD:2 * DH], HPC, 1)
                        qr = qrp.tile([128, HPC, DH], BF16, tag="qr")
                        t1 = tmpp.tile([128, HPC, HD], BF16, tag="t1")
                        t2 = tmpp.tile([128, HPC, HD], BF16, tag="t2")
                        lo = qn[:, :, 0:HD]
                        hi = qn[:, :, HD:DH]
                        nc.vector.tensor_tensor(out=t1[:], in0=hi, in1=sin0[:],
                                                op=ALU.mult)
                        nc.vector.tensor_tensor(out=t2[:], in0=lo, in1=cos0[:],
                                                op=ALU.mult)
                        nc.vector.tensor_tensor(out=qr[:, :, 0:HD],
                                                in0=t2[:], in1=t1[:],
                                                op=ALU.subtract)
                        nc.vector.tensor_tensor(out=t1[:], in0=lo, in1=sin1[:],
                                                op=ALU.mult)
                        nc.vector.tensor_tensor(out=t2[:], in0=hi, in1=cos1[:],
                                                op=ALU.mult)
                        nc.vector.tensor_tensor(out=qr[:, :, HD:DH],
                                                in0=t2[:], in1=t1[:],
                                                op=ALU.add)
                        # one batched xbar transpose per (t, q/k):
                        # [128 tok, 512] -> [128 (hh,dh), 4 j, 128 tok]
                        dst = qT_sb if qk == 0 else kT_sb
                        eng = nc.sync if qk == 0 else nc.scalar
                        eng.dma_start_transpose(
                            out=dst[:, :, t * 128:(t + 1) * 128],
                            in_=qr.rearrange("p h d -> p (h d)"))
                    # ---- v = l1*(v+dv) + pre-scaled residual ----
                    nc.vector.scalar_tensor_tensor(
                        out=v_sb[:, t, :, 0:DH],
                        in0=ps[:, 1024:1536].rearrange("p (h d) -> p h d", h=HPC),
                        scalar=float(l1),
                        in1=vt.rearrange("p (h d) -> p h d", h=HPC),
                        op0=ALU.mult, op1=ALU.add)

            if debug:
                nc.sync.dma_start(out=dbg["d_qT"][:, :, :], in_=qT_sb)
                nc.sync.dma_start(out=dbg["d_kT"][:, :, :], in_=kT_sb)
                nc.sync.dma_start(
                    out=dbg["d_v"][:, :],
                    in_=v_sb.rearrange("p a b c -> p (a b c)"))
            # ---------------- attention + output projection ----------------
            with (
                tc.tile_pool(name="exp", bufs=3) as exp_p,
                tc.tile_pool(name="rcpp", bufs=2) as rcpp,
                tc.tile_pool(name="rcbp", bufs=2) as rcbp,
                tc.tile_pool(name="ostg", bufs=2) as ostg,
                tc.tile_pool(name="stgp", bufs=2) as stgp,
                tc.tile_pool(name="psS", bufs=2, space="PSUM") as psS,
                tc.tile_pool(name="psV", bufs=2, space="PSUM") as psV,
                tc.tile_pool(name="psP", bufs=2, space="PSUM") as psP,
            ):
                def emit_sc(j, qh, kc):
                    sc = psS.tile([128, 1024], F32, tag="sc", name="sc")
                    nc.tensor.matmul(
                        sc[:, 0:512],
                        kT_sb[0:DH, j, kc * 128:(kc + 1) * 128],
                        qT_sb[0:DH, j, qh * 512:(qh + 1) * 512],
                        start=True, stop=True, tile_position=(0, 0))
                    nc.tensor.matmul(
                        sc[:, 512:1024],
                        kT_sb[DH:128, j, kc * 128:(kc + 1) * 128],
                        qT_sb[DH:128, j, qh * 512:(qh + 1) * 512],
                        start=True, stop=True, tile_position=(DH, 0))
                    return sc

                def attn_iter(j, qh):
                    avA = psV.tile([DH + 1, 512], F32, tag="av")
                    avB = psV.tile([DH + 1, 512], F32, tag="av")
                    # software-pipelined by one chunk: sc(kc+1) is emitted
                    # BEFORE av(kc) so the in-order PE queue keeps the next
                    # score matmul ahead of the exp-gated attend matmul and
                    # the scalar engine streams exps back-to-back
                    sc = emit_sc(j, qh, 0)
                    for kc in range(NT):
                        ex = exp_p.tile([128, 1024], BF16, tag="ex")
                        nc.scalar.activation(out=ex[:], in_=sc[:], func=AF.Exp,
                                             scale=1.0 / float(np.sqrt(DH)))
                        if kc < NT - 1:
                            sc = emit_sc(j, qh, kc + 1)
                        nc.tensor.matmul(avA[:], v_sb[:, kc, 2 * j, :],
                                         ex[:, 0:512],
                                         start=(kc == 0), stop=(kc == NT - 1))
                        nc.tensor.matmul(avB[:], v_sb[:, kc, 2 * j + 1, :],
                                         ex[:, 512:1024],
                                         start=(kc == 0), stop=(kc == NT - 1))
                    for hh, av in ((0, avA), (1, avB)):
                        # stage av to SBUF immediately: frees the PSUM bank
                        # for the next iteration, and the approx-reciprocal
                        # custom op misreads PSUM operands on hardware
                        avs = rcpp.tile([DH + 1, 512], F32, tag="avs")
                        nc.vector.tensor_copy(avs[:], av[:])
                        # custom DVE ops and partition_broadcast only honor
                        # partition base 0 on HW: DMA the sum row down first
                        s0 = rcpp.tile([1, 512], F32, tag="s0")
                        nc.sync.dma_start(out=s0[:, :], in_=avs[DH:DH + 1, :])
                        rcp = rcpp.tile([1, 512], F32, tag="rcp")
                        nc.vector.reciprocal_approx_fast(
                            out=rcp[:], in_=s0[:])
                        if debug:
                            ri = (4 * qh + j) * 2 + hh
                            nc.sync.dma_start(
                                out=dbg["d_sum"][ri:ri + 1, :], in_=s0[:, :])
                            nc.sync.dma_start(
                                out=dbg["d_rcp"][ri:ri + 1, :], in_=rcp[:, :])
                        rcb = rcbp.tile([DH, 512], F32, tag="rcb")
                        nc.gpsimd.partition_broadcast(rcb[:], rcp[:])
                        if hh == 0:
                            nc.vector.tensor_tensor(
                                out=outT_sb[0:DH, j, qh * 512:(qh + 1) * 512],
                                in0=avs[0:DH, :], in1=rcb[:], op=ALU.mult)
                        else:
                            # keep DVE partition bases aligned: normalize at
                            # base 0, then DMA the rows into place
                            ob_stg = ostg.tile([DH, 512], BF16, tag="ob")
                            nc.vector.tensor_tensor(
                                out=ob_stg[:], in0=avs[0:DH, :], in1=rcb[:],
                                op=ALU.mult)
                            nc.sync.dma_start(
                                out=outT_sb[DH:128, j, qh * 512:(qh + 1) * 512],
                                in_=ob_stg[:])

                def proj_t(t, on_act=False):
                    stg = stgp.tile([128, C], F32, tag="stg")
                    for oh in range(2):
                        pp = psP.tile([128, 512], F32, tag="pp")
                        for cc in range(4):
                            nc.tensor.matmul(
                                pp[:],
                                outT_sb[:, cc, t * 128:(t + 1) * 128],
                                wproj_sb[:, cc, oh * 512:(oh + 1) * 512],
                                start=(cc == 0), stop=(cc == 3))
                        if on_act:
                            # tail projections: the scalar engine is idle
                            # after the last exp
                            nc.scalar.copy(
                                out=stg[:, oh * 512:(oh + 1) * 512], in_=pp[:])
                        else:
                            nc.vector.tensor_copy(
                                stg[:, oh * 512:(oh + 1) * 512], pp[:])
                    # alternate store queues so the final drain parallelizes
                    seng = nc.sync if t % 2 == 0 else nc.scalar
                    seng.dma_start(out=out[t * 128:(t + 1) * 128, :], in_=stg)

                for j in range(HPC // 2):
                    attn_iter(j, 0)
                for j in range(HPC // 2):
                    attn_iter(j, 1)
                    proj_t(j)
                for t in range(4, NT):
                    proj_t(t, on_act=True)
                if debug:
                    nc.sync.dma_start(out=dbg["d_oT"][:, :, :], in_=outT_sb)

    nc.finalize()
    return nc


def prepare(x, rope, delta_t_emb, v_residual_v1, Wqkv, bqkv, Wdt, bdt,
            qn_g, qn_b, kn_g, kn_b, lamb1, lamb2, Wproj, bproj):
    """Host-side prep: returns (l1, in_maps, bproj)."""
    x = np.asarray(x, np.float32)
    rope = np.ascontiguousarray(np.asarray(rope, np.float32))
    delta_t_emb = np.asarray(delta_t_emb, np.float32)
    v_residual_v1 = np.asarray(v_residual_v1, np.float32)
    Wqkv = np.asarray(Wqkv, np.float32)
    Wdt = np.asarray(Wdt, np.float32)
    Wproj = np.asarray(Wproj, np.float32)
    bias = np.asarray(bqkv, np.float32) + np.asarray(bdt, np.float32)
    l1 = float(np.asarray(lamb1)); l2 = float(np.asarray(lamb2))
    qn_g = np.asarray(qn_g, np.float32); qn_b = np.asarray(qn_b, np.float32)
    kn_g = np.asarray(kn_g, np.float32); kn_b = np.asarray(kn_b, np.float32)

    assert not np.any(bias[:2 * C]), "nonzero q/k bias not supported"
    assert not np.any(qn_b) and not np.any(kn_b), "LN beta not supported"

    # fold LN gamma into the rope tables (per-channel scale commutes with
    # the elementwise rope rotation when gamma is rotate-half symmetric,
    # i.e. gamma[d] == gamma[(d+DH/2)%DH]; the reference's gamma is all-ones)
    HD = DH // 2
    sym_q = np.allclose(qn_g[:HD], qn_g[HD:])
    sym_k = np.allclose(kn_g[:HD], kn_g[HD:])
    assert sym_q and sym_k, "rotate-half-asymmetric LN gamma not supported"
    # fold gamma into the tables: with rotate-half-symmetric gamma, scaling
    # both the sin and cos halves by gamma equals applying gamma before rope
    rope_q = rope * np.concatenate([qn_g, qn_g])[None, :]
    rope_k = rope * np.concatenate([kn_g, kn_g])[None, :]
    same_rope = np.allclose(rope_q, rope_k)

    in_maps = []
    for c in range(8):
        b = c // 2
        g = c % 2
        rsl = slice(g * 512, (g + 1) * 512)
        w_core = np.concatenate([
            np.concatenate([Wqkv[rsl], Wqkv[C:][rsl], Wqkv[2 * C:][rsl]], 0).T,
            np.concatenate([Wdt[rsl], Wdt[C:][rsl], Wdt[2 * C:][rsl]], 0).T,
        ], axis=0).copy()  # [2048, 1536]; cols = q(512) | k(512) | v(512)
        # center q/k weight columns per head: exact identity under LayerNorm
        for h in range(HPC):
            for blk in range(2):
                cs = blk * 512 + h * DH
                w_core[:, cs:cs + DH] -= w_core[:, cs:cs + DH].mean(
                    axis=1, keepdims=True)
        vres_pre = (v_residual_v1[b, g * 8:(g + 1) * 8]
                    .transpose(1, 0, 2).reshape(N, 512) * l2)
        if np.any(bias[2 * C:]):
            vres_pre = vres_pre + l1 * bias[2 * C:][rsl][None, :]
        m = {
            "xdT": np.ascontiguousarray(
                np.concatenate([x[b].T, delta_t_emb[b].T], 0)).astype(NPBF16),
            "w": np.ascontiguousarray(w_core).astype(NPBF16),
            "vres": np.ascontiguousarray(vres_pre, dtype=np.float32),
            "wproj": np.ascontiguousarray(Wproj[:, rsl].T).astype(NPBF16),
            "rope": np.ascontiguousarray(rope_q).astype(NPBF16),
        }
        assert same_rope, "distinct q/k LN gamma not supported"
        in_maps.append(m)
    return l1, in_maps, np.asarray(bproj, np.float32)


_CACHE = {}
_LAST_RES = None


def kernel(x, rope, delta_t_emb, v_residual_v1, Wqkv, bqkv, Wdt, bdt,
           qn_g, qn_b, kn_g, kn_b, lamb1, lamb2, Wproj, bproj):
    l1, in_maps, bproj = prepare(
        x, rope, delta_t_emb, v_residual_v1, Wqkv, bqkv, Wdt, bdt,
        qn_g, qn_b, kn_g, kn_b, lamb1, lamb2, Wproj, bproj)

    dbgf = bool(int(os.environ.get("KERNEL_DEBUG", "0")))
    key = (l1, dbgf)
    if key not in _CACHE:
        _CACHE[key] = build(l1, debug=dbgf)
    nc = _CACHE[key]

    trace = bool(int(os.environ.get("KERNEL_TRACE", "0")))
    res = run_bass_kernel_spmd(nc, in_maps, core_ids=list(range(8)), trace=trace)
    global _LAST_RES
    _LAST_RES = res
    if trace and res.exec_time_ns is not None:
        print(f"HW exec time: {res.exec_time_ns} ns")
        kernel.last_exec_time_ns = res.exec_time_ns
        kernel.last_results = res

    out = np.empty((B, N, C), np.float32)
    for b in range(B):
        out[b] = res.results[2 * b]["out"] + res.results[2 * b + 1]["out"]
    if np.any(bproj):
        out += bproj[None, None, :]
    return out
